# revision 8
# baseline (speedup 1.0000x reference)
"""DiagSSMBlock Trainium2 kernel.

h_t = sum_{k=0..t} a^k * (B^T x_{t-k})  ==  h_t = a * h_{t-1} + s_t, s = B^T x^T.

Strategy: shard T across the 8 cores (1024 steps each + 32-step halo; |a| <=
sqrt(2/1024) ~ 0.044 so a^32 < 1e-43 == 0 in fp32, making slabs exactly
independent).  Host passes x pre-transposed ([H, T_slab]) so the tensor engine
can contract over H with no on-chip transposes; the scan output is returned
channel-major [H, T_slab] and transposed back on host.

Per core: DMA B + xT slab -> 8x K-chunked fp32r matmul accumulation into PSUM
(3 chunks of 352 time-cols) -> tensor_tensor_scan (the SSM recurrence) per
128-channel group -> DMA out.

Perf structure:
- dummy warm-up matmuls lift the PE HAM clock-gate to 2.4 GHz during the ramp
- matmul groups chained on PE so execution order == emission order (scans fire
  promptly, psum slots recycle)
- chunk-outer / group-inner loop keeps input byte-demand under HBM bandwidth
- DMA sizing: small tiles for the startup-critical pieces, big (~0.5MB) tiles
  for the rest -- the HWDGE ring limits DMAs in flight, not bytes, so big
  transfers keep the queues saturated (~400 GB/s) with few issues
"""

import sys

if "/opt/trn_rl_repo" not in sys.path:
    sys.path.insert(0, "/opt/trn_rl_repo")

import numpy as np

T, H = 8192, 1024
NC = 8
P = 128
T_LOC = T // NC            # 1024 output timesteps per core
HALO = 32                  # scan warmup; a^32 == 0 in fp32
W = T_LOC + HALO           # 1056
CH = 352                   # psum chunk width (3 chunks of 352 = 1056)
NCHUNK = W // CH
KQ = H // P                # 8 contraction chunks
G = H // P                 # 8 channel groups
N_WARM = 8                 # dummy matmuls to lift the HAM clock gate

MM_DTYPE = "float32r"      # matmul operand dtype: "float32" (4 cyc/row) or
                           # "float32r" (1 cyc/row at N>=256)

_state = {}


def _build_nc():
    import concourse.bass as bass_mod
    import concourse.tile as tile
    from concourse import bacc, mybir

    mm_dt = getattr(mybir.dt, MM_DTYPE)
    f32 = mybir.dt.float32

    nc = bacc.Bacc("TRN2", target_bir_lowering=False, debug=False, num_devices=NC)
    xt_e = nc.dram_tensor("xt", [H, W], mm_dt, kind="ExternalInput").ap()
    b_e = nc.dram_tensor("b", [H, H], mm_dt, kind="ExternalInput").ap()
    av_e = nc.dram_tensor("av", [P, G], f32, kind="ExternalInput").ap()
    out_e = nc.dram_tensor("out", [H, T_LOC], f32, kind="ExternalOutput").ap()
    flush_e = nc.dram_tensor("warm_flush", [P, 1], f32).ap()

    with tile.TileContext(nc) as tc:
        with (
            tc.tile_pool(name="consts", bufs=1) as consts,
            tc.tile_pool(name="bpool", bufs=1) as bpool,
            tc.tile_pool(name="xpool", bufs=1) as xpool,
            tc.tile_pool(name="hpool", bufs=1) as hpool,
            tc.tile_pool(name="pspool", bufs=7, space="PSUM") as pspool,
            tc.tile_pool(name="warmps", bufs=1, space="PSUM") as warmps,
        ):
            # PE warm-up: dummy fp32 matmuls on a zeroed scratch tile, gated
            # only on a gpsimd memset, so the HAM clock-gate lifts to 2.4 GHz
            # during the input-DMA ramp.
            warm_sb = consts.tile([P, P], f32, tag="warm")
            nc.gpsimd.memset(warm_sb[:], 0.0)
            wps = warmps.tile([P, P], f32)
            last_mm = None
            for i in range(N_WARM):
                last_mm = nc.tensor.matmul(
                    wps[:],
                    warm_sb[:],
                    warm_sb[:],
                    start=(i == 0),
                    stop=(i == N_WARM - 1),
                )

            # a broadcast tiles: av DMA on gpsimd (tiny), memset 1.0 on
            # gpsimd, scaled per-partition on DVE.
            av_sb = consts.tile([P, G], f32, tag="av")
            nc.gpsimd.dma_start(av_sb[:], av_e[:])
            a_bc = []
            for g in range(G):
                t = consts.tile([P, CH], f32, tag=f"abc{g}")
                nc.gpsimd.memset(t[:], 1.0)
                nc.vector.tensor_scalar_mul(t[:], t[:], av_sb[:, g : g + 1])
                a_bc.append(t)

            # Input loads.
            # sync:   xt chunk 0 (8 small), xt chunks 1+2 (8 big), outputs
            # scalar: b cols for groups 0-1 (8 small), b cols 2-7 (8 big)
            xt0_sb = []
            for kq in range(KQ):
                t0 = xpool.tile([P, CH], mm_dt, tag=f"x0_{kq}")
                nc.sync.dma_start(t0[:], xt_e[kq * P : (kq + 1) * P, 0:CH])
                xt0_sb.append(t0)
            b01_sb = []
            for kq in range(KQ):
                bt = bpool.tile([P, 2 * P], mm_dt, tag=f"b01_{kq}")
                nc.scalar.dma_start(bt[:], b_e[kq * P : (kq + 1) * P, 0 : 2 * P])
                b01_sb.append(bt)
            xt12_sb = []
            for kq in range(KQ):
                t12 = xpool.tile([P, 2 * CH], mm_dt, tag=f"x12_{kq}")
                nc.sync.dma_start(
                    t12[:], xt_e[kq * P : (kq + 1) * P, CH : 3 * CH]
                )
                xt12_sb.append(t12)
            br_sb = []
            for kq in range(KQ):
                bt = bpool.tile([P, H - 2 * P], mm_dt, tag=f"br_{kq}")
                nc.scalar.dma_start(bt[:], b_e[kq * P : (kq + 1) * P, 2 * P : H])
                br_sb.append(bt)

            def b_slice(kq, g):
                if g < 2:
                    return b01_sb[kq][:, g * P : (g + 1) * P]
                return br_sb[kq][:, (g - 2) * P : (g - 1) * P]

            def xt_slice(kq, ni):
                if ni == 0:
                    return xt0_sb[kq][:]
                return xt12_sb[kq][:, (ni - 1) * CH : ni * CH]

            # Matmul + scan: chunk-outer / group-inner keeps the input-byte
            # demand curve under HBM bandwidth.
            h_t = []
            for g in range(G):
                hg = hpool.tile([P, W], f32, tag=f"h{g}")
                h_t.append(hg)
            for ni in range(NCHUNK):
                n0 = ni * CH
                for g in range(G):
                    ps = pspool.tile([P, CH], f32)
                    for kq in range(KQ):
                        mm = nc.tensor.matmul(
                            ps[:],
                            b_slice(kq, g),
                            xt_slice(kq, ni),
                            start=(kq == 0),
                            stop=(kq == KQ - 1),
                        )
                        if kq == 0 and last_mm is not None:
                            # pin PE group execution order = emission order so
                            # scans fire promptly and psum slots recycle
                            bass_mod._add_dep_helper(
                                mm.ins, last_mm.ins, False, "PE group order"
                            )
                        last_mm = mm
                    init = 0.0 if ni == 0 else h_t[g][:, n0 - 1 : n0]
                    nc.vector.tensor_tensor_scan(
                        h_t[g][:, n0 : n0 + CH],
                        a_bc[g][:],
                        ps[:],
                        init,
                        op0=mybir.AluOpType.mult,
                        op1=mybir.AluOpType.add,
                    )
                    if ni == NCHUNK - 1:
                        nc.sync.dma_start(
                            out_e[g * P : (g + 1) * P, :], h_t[g][:, HALO:W]
                        )

            # warm-up flush: emitted last so the PSUM read (not legal on
            # gpsimd) sits at the tail of the DVE queue and blocks nothing.
            flush_sb = consts.tile([P, 1], f32, tag="flush")
            nc.vector.tensor_copy(flush_sb[:], wps[:, 0:1])
            nc.gpsimd.dma_start(flush_e[:], flush_sb[:])

    nc.compile()
    return nc


def _get_nc():
    if "nc" not in _state:
        _state["nc"] = _build_nc()
    return _state["nc"]


def _shard_inputs(x_seq, a_diag, b_mat):
    x = np.asarray(x_seq, dtype=np.float32)
    a = np.asarray(a_diag, dtype=np.float32)
    b = np.ascontiguousarray(np.asarray(b_mat, dtype=np.float32))
    x_pad = np.concatenate([np.zeros((HALO, H), np.float32), x], axis=0)
    xT = np.ascontiguousarray(x_pad.T)  # [H, T + HALO]
    av = np.ascontiguousarray(a.reshape(G, P).T)  # [P, G]
    in_maps = []
    for i in range(NC):
        in_maps.append(
            {
                "xt": np.ascontiguousarray(xT[:, i * T_LOC : i * T_LOC + W]),
                "b": b,
                "av": av,
            }
        )
    return in_maps


def kernel(x_seq, a_diag, b_mat):
    from concourse.bass_utils import run_bass_kernel_spmd

    nc = _get_nc()
    in_maps = _shard_inputs(x_seq, a_diag, b_mat)
    res = run_bass_kernel_spmd(nc, in_maps, list(range(NC)))
    _state["last_result"] = res
    out = np.concatenate(
        [np.asarray(res.results[i]["out"]).T for i in range(NC)], axis=0
    )
    return out


# revision 9
# speedup vs baseline: 1.0760x; 1.0760x over previous
"""DiagSSMBlock Trainium2 kernel.

h_t = sum_{k=0..t} a^k * (B^T x_{t-k})  ==  h_t = a * h_{t-1} + s_t, s = B^T x^T.

Strategy: shard T across the 8 cores (1024 steps each + 32-step halo; |a| <=
sqrt(2/1024) ~ 0.044 so a^32 < 1e-43 == 0 in fp32, making slabs exactly
independent).  Host passes x pre-transposed ([H, T_slab]) so the tensor engine
can contract over H with no on-chip transposes; the scan output is returned
channel-major [H, T_slab] and transposed back on host.

Per core: DMA B + xT slab -> 8x K-chunked fp32r matmul accumulation into PSUM
(3 chunks of 352 time-cols) -> tensor_tensor_scan (the SSM recurrence) per
128-channel group -> DMA out.

Perf structure:
- dummy warm-up matmuls lift the PE HAM clock-gate to 2.4 GHz during the ramp
- matmul groups chained on PE so execution order == emission order (scans fire
  promptly, psum slots recycle)
- chunk-outer / group-inner loop keeps input byte-demand under HBM bandwidth
- DMA sizing: small tiles for the startup-critical pieces, big (~0.5MB) tiles
  for the rest -- the HWDGE ring limits DMAs in flight, not bytes, so big
  transfers keep the queues saturated (~400 GB/s) with few issues
"""

import sys

if "/opt/trn_rl_repo" not in sys.path:
    sys.path.insert(0, "/opt/trn_rl_repo")

import numpy as np

T, H = 8192, 1024
NC = 8
P = 128
T_LOC = T // NC            # 1024 output timesteps per core
HALO = 32                  # scan warmup; a^32 == 0 in fp32
W = T_LOC + HALO           # 1056
CH = 352                   # psum chunk width (3 chunks of 352 = 1056)
NCHUNK = W // CH
KQ = H // P                # 8 contraction chunks
G = H // P                 # 8 channel groups
N_WARM = 8                 # dummy matmuls to lift the HAM clock gate

MM_DTYPE = "float32r"      # matmul operand dtype: "float32" (4 cyc/row) or
                           # "float32r" (1 cyc/row at N>=256)

_state = {}


def _build_nc():
    import concourse.bass as bass_mod
    import concourse.tile as tile
    from concourse import bacc, mybir

    mm_dt = getattr(mybir.dt, MM_DTYPE)
    f32 = mybir.dt.float32

    nc = bacc.Bacc("TRN2", target_bir_lowering=False, debug=False, num_devices=NC)
    xt_e = nc.dram_tensor("xt", [H, W], mm_dt, kind="ExternalInput").ap()
    b_e = nc.dram_tensor("b", [H, H], mm_dt, kind="ExternalInput").ap()
    av_e = nc.dram_tensor("av", [P, G], f32, kind="ExternalInput").ap()
    out_e = nc.dram_tensor("out", [H, T_LOC], f32, kind="ExternalOutput").ap()
    flush_e = nc.dram_tensor("warm_flush", [P, 1], f32).ap()

    with tile.TileContext(nc) as tc:
        with (
            tc.tile_pool(name="consts", bufs=1) as consts,
            tc.tile_pool(name="bpool", bufs=1) as bpool,
            tc.tile_pool(name="xpool", bufs=1) as xpool,
            tc.tile_pool(name="hpool", bufs=1) as hpool,
            tc.tile_pool(name="pspool", bufs=7, space="PSUM") as pspool,
            tc.tile_pool(name="warmps", bufs=1, space="PSUM") as warmps,
        ):
            # PE warm-up: dummy fp32 matmuls on a zeroed scratch tile, gated
            # only on a gpsimd memset, so the HAM clock-gate lifts to 2.4 GHz
            # during the input-DMA ramp.
            warm_sb = consts.tile([P, P], f32, tag="warm")
            nc.gpsimd.memset(warm_sb[:], 0.0)
            wps = warmps.tile([P, P], f32)
            last_mm = None
            for i in range(N_WARM):
                last_mm = nc.tensor.matmul(
                    wps[:],
                    warm_sb[:],
                    warm_sb[:],
                    start=(i == 0),
                    stop=(i == N_WARM - 1),
                )

            # a broadcast tiles: av DMA on gpsimd (tiny), memset 1.0 on
            # gpsimd, scaled per-partition on DVE.
            av_sb = consts.tile([P, G], f32, tag="av")
            nc.gpsimd.dma_start(av_sb[:], av_e[:])
            a_bc = []
            for g in range(G):
                t = consts.tile([P, CH], f32, tag=f"abc{g}")
                nc.gpsimd.memset(t[:], 1.0)
                nc.vector.tensor_scalar_mul(t[:], t[:], av_sb[:, g : g + 1])
                a_bc.append(t)

            # Input loads.
            # sync:   xt chunks (3x8 of [128,352]), then the output stores
            # scalar: b fine-grained for groups 0/1, then two 3-group tiles
            xt_sb = [[None] * NCHUNK for _ in range(KQ)]
            for ni in range(NCHUNK):
                n0 = ni * CH
                for kq in range(KQ):
                    xtile = xpool.tile([P, CH], mm_dt, tag=f"x{kq}_{ni}")
                    nc.sync.dma_start(
                        xtile[:], xt_e[kq * P : (kq + 1) * P, n0 : n0 + CH]
                    )
                    xt_sb[kq][ni] = xtile
            bg_sb = [[None] * 4 for _ in range(KQ)]
            for piece, (c0, c1) in enumerate([(0, P), (P, 2 * P), (2 * P, 5 * P), (5 * P, 8 * P)]):
                for kq in range(KQ):
                    bt = bpool.tile([P, c1 - c0], mm_dt, tag=f"b{kq}_{piece}")
                    nc.scalar.dma_start(bt[:], b_e[kq * P : (kq + 1) * P, c0:c1])
                    bg_sb[kq][piece] = bt

            def b_slice(kq, g):
                if g < 2:
                    return bg_sb[kq][g][:]
                if g < 5:
                    return bg_sb[kq][2][:, (g - 2) * P : (g - 1) * P]
                return bg_sb[kq][3][:, (g - 5) * P : (g - 4) * P]

            def xt_slice(kq, ni):
                return xt_sb[kq][ni][:]

            # Matmul + scan: chunk-outer / group-inner keeps the input-byte
            # demand curve under HBM bandwidth.
            h_t = []
            for g in range(G):
                hg = hpool.tile([P, W], f32, tag=f"h{g}")
                h_t.append(hg)
            for g in range(G):
                for ni in range(NCHUNK):
                    n0 = ni * CH
                    ps = pspool.tile([P, CH], f32)
                    for kq in range(KQ):
                        mm = nc.tensor.matmul(
                            ps[:],
                            b_slice(kq, g),
                            xt_slice(kq, ni),
                            start=(kq == 0),
                            stop=(kq == KQ - 1),
                        )
                        if kq == 0 and last_mm is not None:
                            # pin PE group execution order = emission order so
                            # scans fire promptly and psum slots recycle
                            bass_mod._add_dep_helper(
                                mm.ins, last_mm.ins, False, "PE group order"
                            )
                        last_mm = mm
                    init = 0.0 if ni == 0 else h_t[g][:, n0 - 1 : n0]
                    nc.vector.tensor_tensor_scan(
                        h_t[g][:, n0 : n0 + CH],
                        a_bc[g][:],
                        ps[:],
                        init,
                        op0=mybir.AluOpType.mult,
                        op1=mybir.AluOpType.add,
                    )
                    if g < 3:
                        # keep-warm filler during the DMA-bound early phase so
                        # HAM does not re-throttle the PE during supply stalls
                        for i in range(2):
                            mmw = nc.tensor.matmul(
                                wps[:],
                                warm_sb[:],
                                warm_sb[:],
                                start=(i == 0),
                                stop=(i == 1),
                            )
                            bass_mod._add_dep_helper(
                                mmw.ins, last_mm.ins, False, "PE group order"
                            )
                            last_mm = mmw
                nc.sync.dma_start(
                    out_e[g * P : (g + 1) * P, :], h_t[g][:, HALO:W]
                )

            # warm-up flush: emitted last so the PSUM read (not legal on
            # gpsimd) sits at the tail of the DVE queue and blocks nothing.
            flush_sb = consts.tile([P, 1], f32, tag="flush")
            nc.vector.tensor_copy(flush_sb[:], wps[:, 0:1])
            nc.gpsimd.dma_start(flush_e[:], flush_sb[:])

    nc.compile()
    return nc


def _get_nc():
    if "nc" not in _state:
        _state["nc"] = _build_nc()
    return _state["nc"]


def _shard_inputs(x_seq, a_diag, b_mat):
    x = np.asarray(x_seq, dtype=np.float32)
    a = np.asarray(a_diag, dtype=np.float32)
    b = np.ascontiguousarray(np.asarray(b_mat, dtype=np.float32))
    x_pad = np.concatenate([np.zeros((HALO, H), np.float32), x], axis=0)
    xT = np.ascontiguousarray(x_pad.T)  # [H, T + HALO]
    av = np.ascontiguousarray(a.reshape(G, P).T)  # [P, G]
    in_maps = []
    for i in range(NC):
        in_maps.append(
            {
                "xt": np.ascontiguousarray(xT[:, i * T_LOC : i * T_LOC + W]),
                "b": b,
                "av": av,
            }
        )
    return in_maps


def kernel(x_seq, a_diag, b_mat):
    from concourse.bass_utils import run_bass_kernel_spmd

    nc = _get_nc()
    in_maps = _shard_inputs(x_seq, a_diag, b_mat)
    res = run_bass_kernel_spmd(nc, in_maps, list(range(NC)))
    _state["last_result"] = res
    out = np.concatenate(
        [np.asarray(res.results[i]["out"]).T for i in range(NC)], axis=0
    )
    return out
